# revision 1
# baseline (speedup 1.0000x reference)
# Trainium2 Bass kernel for the MEGNet edge model:
#   out = relu(concat([src, dest, edge_attr, u[batch]], 1) @ W1 + b1) @ W2 + b2
#
# Strategy (8 NeuronCores, SPMD, edges sharded contiguously):
#  * All tensors are shipped to the device in a transposed, feature-major
#    layout [128, E_pad] so the PE array can contract over features without
#    any on-chip transposes; the host transposes shards (layout choice made
#    while sharding) and transposes the output back.
#  * comb @ W1 decomposes into src@W1a + dest@W1b + edge_attr@W1c +
#    u[batch]@W1d.  The u[batch] term plus b1 is folded into a per-group
#    table z = u @ W1d + b1 [G, 128]; since batch is sorted, each 512-edge
#    tile only spans a few consecutive groups, so z[batch] is applied with
#    one extra small matmul per tile: lhsT = the k_s candidate z-rows of
#    that tile, rhs = a one-hot selection matrix built on the host.
#  * PSUM accumulates the 4 matmul terms; ScalarE applies ReLU (PSUM->SBUF);
#    the second matmul uses W2; VectorE adds b2 (per-partition vector).
import os
import numpy as np

N_CORES = 8
P = 128      # feature dim == SBUF partitions
TILE = 512   # edges per matmul tile (one PSUM bank of fp32)
CH = 7       # matmul tiles per DMA chunk (98 tiles = 14 chunks of 7)

# Matmul operand dtype: "f32" (exact, 2 HW passes), "f32r" (same fp32 bits,
# single-pass reduced-precision multiply), "bf16" (half DMA traffic too).
MM_DTYPE = os.environ.get("KERNEL_MM_DTYPE", "f32r")

_prog_cache = {}


def _np_mm_dtype():
    if MM_DTYPE == "bf16":
        import ml_dtypes
        return ml_dtypes.bfloat16
    return np.float32


def _build_program(T, k_s):
    import concourse.bacc as bacc
    import concourse.tile as tile
    from concourse import mybir

    f32 = mybir.dt.float32
    mdt = {"f32": mybir.dt.float32, "f32r": mybir.dt.float32r,
           "bf16": mybir.dt.bfloat16}[MM_DTYPE]
    Relu = mybir.ActivationFunctionType.Relu
    Epad = T * TILE

    nc = bacc.Bacc("TRN2", target_bir_lowering=False, debug=False,
                   num_devices=N_CORES)
    srcT = nc.dram_tensor("srcT", [P, Epad], mdt, kind="ExternalInput")
    destT = nc.dram_tensor("destT", [P, Epad], mdt, kind="ExternalInput")
    eaT = nc.dram_tensor("eaT", [P, Epad], mdt, kind="ExternalInput")
    w1d = nc.dram_tensor("w1", [3 * P, P], mdt, kind="ExternalInput")
    w2d = nc.dram_tensor("w2", [P, P], mdt, kind="ExternalInput")
    b2d = nc.dram_tensor("b2c", [P, 1], f32, kind="ExternalInput")
    seld = nc.dram_tensor("sel", [k_s, Epad], mdt, kind="ExternalInput")
    zwd = nc.dram_tensor("zw", [k_s, T * P], mdt, kind="ExternalInput")
    outT = nc.dram_tensor("outT", [P, Epad], f32, kind="ExternalOutput")

    assert T % CH == 0
    n_chunks = T // CH
    CW = CH * TILE  # chunk width in edges

    with tile.TileContext(nc) as tc:
        with (
            tc.tile_pool(name="const", bufs=1) as constp,
            tc.tile_pool(name="inp", bufs=2) as inp,
            tc.tile_pool(name="hp", bufs=4) as hp,
            tc.tile_pool(name="outp", bufs=2) as outp,
            tc.tile_pool(name="psum", bufs=3, space="PSUM") as psum,
        ):
            w1a = constp.tile([P, P], mdt, tag="w1a", name="w1a")
            w1b = constp.tile([P, P], mdt, tag="w1b", name="w1b")
            w1c = constp.tile([P, P], mdt, tag="w1c", name="w1c")
            w2s = constp.tile([P, P], mdt, tag="w2s", name="w2s")
            b2s = constp.tile([P, 1], f32, tag="b2s", name="b2s")
            zws = constp.tile([k_s, T * P], mdt, tag="zws", name="zws")
            nc.gpsimd.dma_start(zws[:], zwd[:])
            nc.sync.dma_start(w1a[:], w1d[0:P, :])
            nc.sync.dma_start(w1b[:], w1d[P:2 * P, :])
            nc.sync.dma_start(w1c[:], w1d[2 * P:3 * P, :])
            nc.sync.dma_start(w2s[:], w2d[:])
            nc.sync.dma_start(b2s[:], b2d[:])

            for c in range(n_chunks):
                base = c * CW
                st = inp.tile([P, CW], mdt, tag="src", name=f"st{c}")
                nc.sync.dma_start(st[:], srcT[:, base:base + CW])
                dt = inp.tile([P, CW], mdt, tag="dest", name=f"dt{c}")
                nc.gpsimd.dma_start(dt[:], destT[:, base:base + CW])
                et = inp.tile([P, CW], mdt, tag="ea", name=f"et{c}")
                nc.sync.dma_start(et[:], eaT[:, base:base + CW])
                slt = inp.tile([k_s, CW], mdt, tag="sel", name=f"slt{c}")
                nc.gpsimd.dma_start(slt[:], seld[:, base:base + CW])
                ot = outp.tile([P, CW], f32, tag="o", name=f"ot{c}")

                for tl in range(CH):
                    t = c * CH + tl
                    cs = slice(tl * TILE, (tl + 1) * TILE)
                    p1 = psum.tile([P, TILE], f32, tag="p1", name=f"p1_{t}")
                    nc.tensor.matmul(p1[:], w1a[:], st[:, cs],
                                     start=True, stop=False)
                    nc.tensor.matmul(p1[:], w1b[:], dt[:, cs],
                                     start=False, stop=False)
                    nc.tensor.matmul(p1[:], w1c[:], et[:, cs],
                                     start=False, stop=False)
                    for j0 in range(0, k_s, P):
                        j1 = min(j0 + P, k_s)
                        nc.tensor.matmul(p1[:],
                                         zws[j0:j1, t * P:(t + 1) * P],
                                         slt[j0:j1, cs],
                                         start=False, stop=(j1 == k_s))
                    h = hp.tile([P, TILE], mdt, tag="h", name=f"h{t}")
                    nc.scalar.activation(h[:], p1[:], Relu)
                    p2 = psum.tile([P, TILE], f32, tag="p2", name=f"p2_{t}")
                    nc.tensor.matmul(p2[:], w2s[:], h[:], start=True, stop=True)
                    nc.vector.tensor_scalar_add(ot[:, cs], p2[:], b2s[:])

                if c % 2 == 0:
                    nc.sync.dma_start(outT[:, base:base + CW], ot[:])
                else:
                    nc.gpsimd.dma_start(outT[:, base:base + CW], ot[:])

    nc.compile()
    return nc


def _get_program(T, k_s):
    key = (T, k_s)
    if key not in _prog_cache:
        _prog_cache[key] = _build_program(T, k_s)
    return _prog_cache[key]


def _install_profile_shim():
    """Optional: enable NTFF profiling under axon (KERNEL_PROFILE=1)."""
    import sys, types
    if "antenv.axon_hooks" not in sys.modules:
        mod = types.ModuleType("antenv.axon_hooks")
        mod._hook = None
        mod.set_axon_ntff_profile_hook = lambda h: setattr(mod, "_hook", h)
        mod.get_axon_ntff_profile_hook = lambda: mod._hook
        sys.modules["antenv.axon_hooks"] = mod
        try:
            import antenv
            antenv.axon_hooks = mod
        except ImportError:
            pass
        try:
            from trn_agent_boot.trn_boot import _ntff_profile_via_ctypes
            mod.set_axon_ntff_profile_hook(
                _ntff_profile_via_ctypes("/opt/axon/libaxon_pjrt.so"))
        except Exception:
            pass
    import concourse.bass_utils as bass_utils
    bass_utils.upload_artifacts = lambda tmpdir: tmpdir


def kernel(src, dest, edge_attr, u, batch, W1, b1, W2, b2):
    src = np.asarray(src, dtype=np.float32)
    dest = np.asarray(dest, dtype=np.float32)
    edge_attr = np.asarray(edge_attr, dtype=np.float32)
    u = np.asarray(u, dtype=np.float32)
    W1 = np.asarray(W1, dtype=np.float32)
    b1 = np.asarray(b1, dtype=np.float32)
    W2 = np.asarray(W2, dtype=np.float32)
    b2 = np.asarray(b2, dtype=np.float32)
    b = np.asarray(batch).astype(np.int64)

    E, D = src.shape
    G = u.shape[0]
    assert D == P and E % N_CORES == 0
    E0 = E // N_CORES
    CW = CH * TILE
    Epad = ((E0 + CW - 1) // CW) * CW
    T = Epad // TILE

    # Fold u[batch] @ W1d + b1 into a per-group table (tiny: G x D).
    z = (u @ W1[3 * D:4 * D] + b1).astype(np.float32)  # [G, D]

    # Per-core: tile-local group offsets for the z-selection matmul.
    g0s, js = [], []
    k_s = 1
    for c in range(N_CORES):
        bc = b[c * E0:(c + 1) * E0]
        bp = np.concatenate([bc, np.full(Epad - E0, bc[-1], dtype=np.int64)])
        per_tile = bp.reshape(T, TILE)
        g0 = per_tile.min(axis=1)                 # [T]
        j = bp - np.repeat(g0, TILE)              # [Epad], >= 0
        g0s.append(g0)
        js.append(j)
        k_s = max(k_s, int(j.max()) + 1)

    mmdt = _np_mm_dtype()
    in_maps = []
    w1_in = np.ascontiguousarray(W1[:3 * D]).astype(mmdt)
    w2_in = np.ascontiguousarray(W2).astype(mmdt)
    b2_in = np.ascontiguousarray(b2.reshape(P, 1))
    for c in range(N_CORES):
        sl = slice(c * E0, (c + 1) * E0)

        def tr(x):
            out = np.zeros((P, Epad), dtype=mmdt)
            out[:, :E0] = x[sl].T.astype(mmdt)
            return out

        selc = np.zeros((k_s, Epad), dtype=mmdt)
        selc[js[c], np.arange(Epad)] = 1.0
        selc[:, E0:] = 0.0  # pad edges contribute nothing
        gidx = np.clip(g0s[c][:, None] + np.arange(k_s)[None, :], 0, G - 1)
        zwc = np.ascontiguousarray(
            z[gidx].transpose(1, 0, 2).reshape(k_s, T * P)).astype(mmdt)
        in_maps.append({
            "srcT": tr(src), "destT": tr(dest), "eaT": tr(edge_attr),
            "w1": w1_in, "w2": w2_in, "b2c": b2_in,
            "sel": selc, "zw": zwc,
        })

    profile = os.environ.get("KERNEL_PROFILE", "") == "1"
    if profile:
        _install_profile_shim()

    nc = _get_program(T, k_s)
    from concourse.bass_utils import run_bass_kernel_spmd
    kwargs = {}
    if profile:
        kwargs["trace"] = True
        if os.environ.get("KERNEL_PROFILE_ALL", "") == "1":
            kwargs["trace_cores"] = list(range(N_CORES))
    res = run_bass_kernel_spmd(nc, in_maps, core_ids=list(range(N_CORES)),
                               **kwargs)
    if profile and res.exec_time_ns is not None:
        with open("/tmp/kernel_exec_ns.txt", "w") as f:
            f.write(str(res.exec_time_ns))
        print(f"HW exec time: {res.exec_time_ns} ns")

    out = np.empty((E, P), dtype=np.float32)
    for c in range(N_CORES):
        out[c * E0:(c + 1) * E0] = res.results[c]["outT"][:, :E0].T
    return out



# revision 4
# speedup vs baseline: 1.9235x; 1.9235x over previous
# Trainium2 Bass kernel for the MEGNet edge model:
#   out = relu(concat([src, dest, edge_attr, u[batch]], 1) @ W1 + b1) @ W2 + b2
#
# Strategy (8 NeuronCores, SPMD, edges sharded contiguously):
#  * All tensors are shipped to the device in a transposed, feature-major
#    layout [128, E_pad] so the PE array can contract over features without
#    any on-chip transposes; the host transposes shards (layout choice made
#    while sharding) and transposes the output back.
#  * comb @ W1 decomposes into src@W1a + dest@W1b + edge_attr@W1c +
#    u[batch]@W1d.  The u[batch] term plus b1 is folded into a per-group
#    table z = u @ W1d + b1 [G, 128]; since batch is sorted, each 512-edge
#    tile only spans a few consecutive groups, so z[batch] is applied with
#    one extra small matmul per tile: lhsT = the k_s candidate z-rows of
#    that tile, rhs = a one-hot selection matrix built on the host.
#  * PSUM accumulates the 4 matmul terms; ScalarE applies ReLU (PSUM->SBUF);
#    the second matmul uses W2; VectorE adds b2 (per-partition vector).
import os
import numpy as np

N_CORES = 8
P = 128      # feature dim == SBUF partitions
TILE = 512   # edges per matmul tile (one PSUM bank of fp32)
CH = int(os.environ.get("KERNEL_CH", "14"))  # matmul tiles per DMA chunk

# Matmul operand dtype: "f32" (exact, 2 HW passes), "f32r" (same fp32 bits,
# single-pass reduced-precision multiply), "bf16" (half DMA traffic too).
MM_DTYPE = os.environ.get("KERNEL_MM_DTYPE", "bf16")
# Output DRAM dtype: bf16 halves the writeback traffic; host converts to f32.
OUT_BF16 = os.environ.get("KERNEL_OUT_BF16", "1") == "1"

_prog_cache = {}


def _np_mm_dtype():
    if MM_DTYPE == "bf16":
        import ml_dtypes
        return ml_dtypes.bfloat16
    return np.float32


def _build_program(T, k_s):
    import concourse.bacc as bacc
    import concourse.tile as tile
    from concourse import mybir

    f32 = mybir.dt.float32
    mdt = {"f32": mybir.dt.float32, "f32r": mybir.dt.float32r,
           "bf16": mybir.dt.bfloat16}[MM_DTYPE]
    odt = mybir.dt.bfloat16 if OUT_BF16 else f32
    Relu = mybir.ActivationFunctionType.Relu
    Epad = T * TILE

    nc = bacc.Bacc("TRN2", target_bir_lowering=False, debug=False,
                   num_devices=N_CORES)
    srcT = nc.dram_tensor("srcT", [P, Epad], mdt, kind="ExternalInput")
    destT = nc.dram_tensor("destT", [P, Epad], mdt, kind="ExternalInput")
    eaT = nc.dram_tensor("eaT", [P, Epad], mdt, kind="ExternalInput")
    w1d = nc.dram_tensor("w1", [3 * P, P], mdt, kind="ExternalInput")
    w2d = nc.dram_tensor("w2", [P, P], mdt, kind="ExternalInput")
    b2d = nc.dram_tensor("b2c", [P, 1], f32, kind="ExternalInput")
    seld = nc.dram_tensor("sel", [k_s, Epad], mdt, kind="ExternalInput")
    zwd = nc.dram_tensor("zw", [k_s, T * P], mdt, kind="ExternalInput")
    outT = nc.dram_tensor("outT", [P, Epad], odt, kind="ExternalOutput")

    assert T % CH == 0
    n_chunks = T // CH
    CW = CH * TILE  # chunk width in edges

    with tile.TileContext(nc) as tc:
        with (
            tc.tile_pool(name="const", bufs=1) as constp,
            tc.tile_pool(name="inp", bufs=2) as inp,
            tc.tile_pool(name="hp", bufs=4) as hp,
            tc.tile_pool(name="outp", bufs=2) as outp,
            tc.tile_pool(name="psum", bufs=3, space="PSUM") as psum,
        ):
            w1a = constp.tile([P, P], mdt, tag="w1a", name="w1a")
            w1b = constp.tile([P, P], mdt, tag="w1b", name="w1b")
            w1c = constp.tile([P, P], mdt, tag="w1c", name="w1c")
            w2s = constp.tile([P, P], mdt, tag="w2s", name="w2s")
            b2s = constp.tile([P, 1], f32, tag="b2s", name="b2s")
            zws = constp.tile([k_s, T * P], mdt, tag="zws", name="zws")
            nc.gpsimd.dma_start(zws[:], zwd[:])
            nc.gpsimd.dma_start(w1a[:], w1d[0:P, :])
            nc.gpsimd.dma_start(w1b[:], w1d[P:2 * P, :])
            nc.gpsimd.dma_start(w1c[:], w1d[2 * P:3 * P, :])
            nc.gpsimd.dma_start(w2s[:], w2d[:])
            nc.gpsimd.dma_start(b2s[:], b2d[:])

            for c in range(n_chunks):
                base = c * CW
                st = inp.tile([P, CW], mdt, tag="src", name=f"st{c}")
                nc.sync.dma_start(st[:], srcT[:, base:base + CW])
                dt = inp.tile([P, CW], mdt, tag="dest", name=f"dt{c}")
                nc.scalar.dma_start(dt[:], destT[:, base:base + CW])
                et = inp.tile([P, CW], mdt, tag="ea", name=f"et{c}")
                nc.sync.dma_start(et[:], eaT[:, base:base + CW])
                slt = inp.tile([k_s, CW], mdt, tag="sel", name=f"slt{c}")
                nc.gpsimd.dma_start(slt[:], seld[:, base:base + CW])
                ot = outp.tile([P, CW], odt, tag="o", name=f"ot{c}")

                for tl in range(CH):
                    t = c * CH + tl
                    cs = slice(tl * TILE, (tl + 1) * TILE)
                    p1 = psum.tile([P, TILE], f32, tag="p1", name=f"p1_{t}")
                    nc.tensor.matmul(p1[:], w1a[:], st[:, cs],
                                     start=True, stop=False)
                    nc.tensor.matmul(p1[:], w1b[:], dt[:, cs],
                                     start=False, stop=False)
                    nc.tensor.matmul(p1[:], w1c[:], et[:, cs],
                                     start=False, stop=False)
                    for j0 in range(0, k_s, P):
                        j1 = min(j0 + P, k_s)
                        nc.tensor.matmul(p1[:],
                                         zws[j0:j1, t * P:(t + 1) * P],
                                         slt[j0:j1, cs],
                                         start=False, stop=(j1 == k_s))
                    h = hp.tile([P, TILE], mdt, tag="h", name=f"h{t}")
                    nc.scalar.activation(h[:], p1[:], Relu)
                    p2 = psum.tile([P, TILE], f32, tag="p2", name=f"p2_{t}")
                    nc.tensor.matmul(p2[:], w2s[:], h[:], start=True, stop=True)
                    nc.vector.tensor_scalar_add(ot[:, cs], p2[:], b2s[:])

                if c % 2 == 0:
                    nc.scalar.dma_start(outT[:, base:base + CW], ot[:])
                else:
                    nc.sync.dma_start(outT[:, base:base + CW], ot[:])

    nc.compile()
    return nc


def _get_program(T, k_s):
    key = (T, k_s)
    if key not in _prog_cache:
        _prog_cache[key] = _build_program(T, k_s)
    return _prog_cache[key]


def _install_profile_shim():
    """Optional: enable NTFF profiling under axon (KERNEL_PROFILE=1)."""
    import sys, types
    if "antenv.axon_hooks" not in sys.modules:
        mod = types.ModuleType("antenv.axon_hooks")
        mod._hook = None
        mod.set_axon_ntff_profile_hook = lambda h: setattr(mod, "_hook", h)
        mod.get_axon_ntff_profile_hook = lambda: mod._hook
        sys.modules["antenv.axon_hooks"] = mod
        try:
            import antenv
            antenv.axon_hooks = mod
        except ImportError:
            pass
        try:
            from trn_agent_boot.trn_boot import _ntff_profile_via_ctypes
            mod.set_axon_ntff_profile_hook(
                _ntff_profile_via_ctypes("/opt/axon/libaxon_pjrt.so"))
        except Exception:
            pass
    import concourse.bass_utils as bass_utils
    bass_utils.upload_artifacts = lambda tmpdir: tmpdir


def kernel(src, dest, edge_attr, u, batch, W1, b1, W2, b2):
    src = np.asarray(src, dtype=np.float32)
    dest = np.asarray(dest, dtype=np.float32)
    edge_attr = np.asarray(edge_attr, dtype=np.float32)
    u = np.asarray(u, dtype=np.float32)
    W1 = np.asarray(W1, dtype=np.float32)
    b1 = np.asarray(b1, dtype=np.float32)
    W2 = np.asarray(W2, dtype=np.float32)
    b2 = np.asarray(b2, dtype=np.float32)
    b = np.asarray(batch).astype(np.int64)

    E, D = src.shape
    G = u.shape[0]
    assert D == P and E % N_CORES == 0
    E0 = E // N_CORES
    CW = CH * TILE
    Epad = ((E0 + CW - 1) // CW) * CW
    T = Epad // TILE

    # Fold u[batch] @ W1d + b1 into a per-group table (tiny: G x D).
    z = (u @ W1[3 * D:4 * D] + b1).astype(np.float32)  # [G, D]

    # Per-core: tile-local group offsets for the z-selection matmul.
    g0s, js = [], []
    k_s = 1
    for c in range(N_CORES):
        bc = b[c * E0:(c + 1) * E0]
        bp = np.concatenate([bc, np.full(Epad - E0, bc[-1], dtype=np.int64)])
        per_tile = bp.reshape(T, TILE)
        g0 = per_tile.min(axis=1)                 # [T]
        j = bp - np.repeat(g0, TILE)              # [Epad], >= 0
        g0s.append(g0)
        js.append(j)
        k_s = max(k_s, int(j.max()) + 1)

    mmdt = _np_mm_dtype()
    in_maps = []
    w1_in = np.ascontiguousarray(W1[:3 * D]).astype(mmdt)
    w2_in = np.ascontiguousarray(W2).astype(mmdt)
    b2_in = np.ascontiguousarray(b2.reshape(P, 1))
    for c in range(N_CORES):
        sl = slice(c * E0, (c + 1) * E0)

        def tr(x):
            out = np.zeros((P, Epad), dtype=mmdt)
            out[:, :E0] = x[sl].T.astype(mmdt)
            return out

        selc = np.zeros((k_s, Epad), dtype=mmdt)
        selc[js[c], np.arange(Epad)] = 1.0
        selc[:, E0:] = 0.0  # pad edges contribute nothing
        gidx = np.clip(g0s[c][:, None] + np.arange(k_s)[None, :], 0, G - 1)
        zwc = np.ascontiguousarray(
            z[gidx].transpose(1, 0, 2).reshape(k_s, T * P)).astype(mmdt)
        in_maps.append({
            "srcT": tr(src), "destT": tr(dest), "eaT": tr(edge_attr),
            "w1": w1_in, "w2": w2_in, "b2c": b2_in,
            "sel": selc, "zw": zwc,
        })

    profile = os.environ.get("KERNEL_PROFILE", "") == "1"
    if profile:
        _install_profile_shim()

    nc = _get_program(T, k_s)
    from concourse.bass_utils import run_bass_kernel_spmd
    kwargs = {}
    if profile:
        kwargs["trace"] = True
        if os.environ.get("KERNEL_PROFILE_ALL", "") == "1":
            kwargs["trace_cores"] = list(range(N_CORES))
    res = run_bass_kernel_spmd(nc, in_maps, core_ids=list(range(N_CORES)),
                               **kwargs)
    if profile and res.exec_time_ns is not None:
        with open("/tmp/kernel_exec_ns.txt", "w") as f:
            f.write(str(res.exec_time_ns))
        print(f"HW exec time: {res.exec_time_ns} ns")

    out = np.empty((E, P), dtype=np.float32)
    for c in range(N_CORES):
        out[c * E0:(c + 1) * E0] = \
            res.results[c]["outT"][:, :E0].T.astype(np.float32)
    return out



# revision 6
# speedup vs baseline: 2.1841x; 1.1354x over previous
# Trainium2 Bass kernel for the MEGNet edge model:
#   out = relu(concat([src, dest, edge_attr, u[batch]], 1) @ W1 + b1) @ W2 + b2
#
# Strategy (8 NeuronCores, SPMD, edges sharded contiguously):
#  * All tensors are shipped in a transposed, feature-major layout [128, E_pad]
#    so the PE array contracts over features with no on-chip transposes; the
#    host transposes shards on the way in and the output on the way out.
#  * comb @ W1 decomposes into src@W1a + dest@W1b + edge_attr@W1c +
#    u[batch]@W1d.  The u[batch] term plus b1 folds into a per-group table
#    z = u @ W1d + b1 [G, 128]; batch is sorted, so each 512-edge tile spans
#    only a few consecutive groups and z[batch] is applied with one small
#    matmul per tile (lhsT = the k_s candidate z-rows, rhs = a one-hot
#    selection matrix built on the host).
#  * Traffic is the roofline: dest/edge_attr/output ship as bf16, src and the
#    one-hot sel matrix as fp8e4 (sel is 0/1, exact in fp8).  Total rel err
#    ~1.5e-2 vs the 2e-2 gate (validated numerically against the reference).
#  * The W2 matmul of tile t is issued after tile t+1's accumulation matmuls
#    so the PE never stalls waiting for the ReLU (keeps the PE p-state ramp
#    going: TRN2 PE only reaches 2.4 GHz after ~3us of continuous execution).
import os
import numpy as np

N_CORES = 8
P = 128      # feature dim == SBUF partitions
TILE = 512   # edges per matmul tile (one PSUM bank of fp32)
CH = 14      # max matmul tiles per DMA chunk
# chunk sizes: smaller leading chunks let compute start sooner
CHUNK_SIZES = [7, 7] + [14] * 6      # sums to 98 tiles

MM_DTYPE = os.environ.get("KERNEL_MM_DTYPE", "bf16")
SRC_FP8 = os.environ.get("KERNEL_SRC_FP8", "1") == "1"
OUT_BF16 = os.environ.get("KERNEL_OUT_BF16", "1") == "1"

_prog_cache = {}


def _np_dtypes():
    import ml_dtypes
    mm = {"f32": np.float32, "f32r": np.float32,
          "bf16": ml_dtypes.bfloat16}[MM_DTYPE]
    sdt = ml_dtypes.float8_e4m3 if SRC_FP8 else mm
    return mm, sdt


def _build_program(T, k_s):
    import concourse.bacc as bacc
    import concourse.tile as tile
    from concourse import mybir

    f32 = mybir.dt.float32
    mdt = {"f32": mybir.dt.float32, "f32r": mybir.dt.float32r,
           "bf16": mybir.dt.bfloat16}[MM_DTYPE]
    sdt = mybir.dt.float8e4 if SRC_FP8 else mdt
    odt = mybir.dt.bfloat16 if OUT_BF16 else f32
    Relu = mybir.ActivationFunctionType.Relu
    Epad = T * TILE

    nc = bacc.Bacc("TRN2", target_bir_lowering=False, debug=False,
                   num_devices=N_CORES)
    srcT = nc.dram_tensor("srcT", [P, Epad], sdt, kind="ExternalInput")
    destT = nc.dram_tensor("destT", [P, Epad], mdt, kind="ExternalInput")
    eaT = nc.dram_tensor("eaT", [P, Epad], mdt, kind="ExternalInput")
    w1d = nc.dram_tensor("w1", [3 * P, P], mdt, kind="ExternalInput")
    w2d = nc.dram_tensor("w2", [P, P], mdt, kind="ExternalInput")
    b2d = nc.dram_tensor("b2c", [P, 1], f32, kind="ExternalInput")
    seld = nc.dram_tensor("sel", [k_s, Epad], sdt, kind="ExternalInput")
    zwd = nc.dram_tensor("zw", [k_s, T * P], mdt, kind="ExternalInput")
    outT = nc.dram_tensor("outT", [P, Epad], odt, kind="ExternalOutput")

    assert sum(CHUNK_SIZES) == T
    CW = CH * TILE  # max chunk width in edges (pool slot size)

    with tile.TileContext(nc) as tc:
        with (
            tc.tile_pool(name="const", bufs=1) as constp,
            tc.tile_pool(name="inp", bufs=2) as inp,
            tc.tile_pool(name="hp", bufs=6) as hp,
            tc.tile_pool(name="outp", bufs=4) as outp,
            tc.tile_pool(name="ps1", bufs=4, space="PSUM") as ps1,
            tc.tile_pool(name="ps2", bufs=4, space="PSUM") as ps2,
        ):
            w1a = constp.tile([P, P], mdt, tag="w1a", name="w1a")
            w1b = constp.tile([P, P], mdt, tag="w1b", name="w1b")
            w1c = constp.tile([P, P], mdt, tag="w1c", name="w1c")
            w2s = constp.tile([P, P], mdt, tag="w2s", name="w2s")
            b2s = constp.tile([P, 1], f32, tag="b2s", name="b2s")
            zws = constp.tile([k_s, T * P], mdt, tag="zws", name="zws")
            nc.gpsimd.dma_start(zws[:], zwd[:])
            nc.gpsimd.dma_start(w1a[:], w1d[0:P, :])
            nc.gpsimd.dma_start(w1b[:], w1d[P:2 * P, :])
            nc.gpsimd.dma_start(w1c[:], w1d[2 * P:3 * P, :])
            nc.gpsimd.dma_start(w2s[:], w2d[:])
            nc.gpsimd.dma_start(b2s[:], b2d[:])

            # Software-pipelined tile stream: accumulate tile t, then issue
            # the W2 matmul of tile t-1 (whose ReLU ran during tile t's MMs).
            # Output DMAs are likewise delayed one tile so the last vector
            # add into an output tile lands before its DMA is issued.
            pend = None      # (h_tile, ot_tile, col_slice_in_ot, tile_idx)
            pend_out = None  # (dram_lo, dram_hi, ot_tile, ow, queue_parity)

            def flush_pending():
                nonlocal pend
                if pend is None:
                    return
                h, ot, ocs, ti = pend
                p2 = ps2.tile([P, TILE], f32, tag="p2", name=f"p2_{ti}")
                nc.tensor.matmul(p2[:], w2s[:], h[:], start=True, stop=True)
                nc.vector.tensor_scalar_add(ot[:, ocs], p2[:], b2s[:])
                pend = None

            def issue_pend_out():
                nonlocal pend_out
                if pend_out is None:
                    return
                lo, hi, ot, ow, par = pend_out
                eng = nc.scalar if par == 0 else nc.sync
                eng.dma_start(outT[:, lo:hi], ot[:, :ow])
                pend_out = None

            t = 0
            for ci, csz in enumerate(CHUNK_SIZES):
                base = t * TILE
                cw = csz * TILE
                st = inp.tile([P, CW], sdt, tag="src", name=f"st{ci}")
                nc.sync.dma_start(st[:, :cw], srcT[:, base:base + cw])
                dt = inp.tile([P, CW], mdt, tag="dest", name=f"dt{ci}")
                nc.sync.dma_start(dt[:, :cw], destT[:, base:base + cw])
                et = inp.tile([P, CW], mdt, tag="ea", name=f"et{ci}")
                nc.scalar.dma_start(et[:, :cw], eaT[:, base:base + cw])
                slt = inp.tile([k_s, CW], sdt, tag="sel", name=f"slt{ci}")
                nc.gpsimd.dma_start(slt[:, :cw], seld[:, base:base + cw])

                # output tiles per half-chunk for a short drain tail
                nho = (csz + 6) // 7  # 1 for 7-tile chunks, 2 for 14
                for ho in range(nho):
                    o0 = ho * 7 * TILE
                    ow = min(7 * TILE, cw - o0)
                    ot = outp.tile([P, 7 * TILE], odt, tag="o",
                                   name=f"ot{ci}_{ho}")

                    for tl in range(ho * 7, min(ho * 7 + 7, csz)):
                        cs = slice(tl * TILE, (tl + 1) * TILE)
                        p1 = ps1.tile([P, TILE], f32, tag="p1",
                                      name=f"p1_{t}")
                        nc.tensor.matmul(p1[:], w1a[:], st[:, cs],
                                         start=True, stop=False)
                        nc.tensor.matmul(p1[:], w1b[:], dt[:, cs],
                                         start=False, stop=False)
                        nc.tensor.matmul(p1[:], w1c[:], et[:, cs],
                                         start=False, stop=False)
                        for j0 in range(0, k_s, P):
                            j1 = min(j0 + P, k_s)
                            nc.tensor.matmul(p1[:],
                                             zws[j0:j1, t * P:(t + 1) * P],
                                             slt[j0:j1, cs],
                                             start=False, stop=(j1 == k_s))
                        flush_pending()
                        if tl == ho * 7:
                            issue_pend_out()
                        h = hp.tile([P, TILE], mdt, tag="h", name=f"h{t}")
                        nc.scalar.activation(h[:], p1[:], Relu)
                        ocs = slice(tl * TILE - o0, (tl + 1) * TILE - o0)
                        pend = (h, ot, ocs, t)
                        t += 1

                    pend_out = (base + o0, base + o0 + ow, ot, ow, ci % 2)
            flush_pending()
            issue_pend_out()

    nc.compile()
    return nc


def _get_program(T, k_s):
    key = (T, k_s)
    if key not in _prog_cache:
        _prog_cache[key] = _build_program(T, k_s)
    return _prog_cache[key]


def _install_profile_shim():
    """Optional: enable NTFF profiling under axon (KERNEL_PROFILE=1)."""
    import sys, types
    if "antenv.axon_hooks" not in sys.modules:
        mod = types.ModuleType("antenv.axon_hooks")
        mod._hook = None
        mod.set_axon_ntff_profile_hook = lambda h: setattr(mod, "_hook", h)
        mod.get_axon_ntff_profile_hook = lambda: mod._hook
        sys.modules["antenv.axon_hooks"] = mod
        try:
            import antenv
            antenv.axon_hooks = mod
        except ImportError:
            pass
        try:
            from trn_agent_boot.trn_boot import _ntff_profile_via_ctypes
            mod.set_axon_ntff_profile_hook(
                _ntff_profile_via_ctypes("/opt/axon/libaxon_pjrt.so"))
        except Exception:
            pass
    import concourse.bass_utils as bass_utils
    bass_utils.upload_artifacts = lambda tmpdir: tmpdir


def kernel(src, dest, edge_attr, u, batch, W1, b1, W2, b2):
    src = np.asarray(src, dtype=np.float32)
    dest = np.asarray(dest, dtype=np.float32)
    edge_attr = np.asarray(edge_attr, dtype=np.float32)
    u = np.asarray(u, dtype=np.float32)
    W1 = np.asarray(W1, dtype=np.float32)
    b1 = np.asarray(b1, dtype=np.float32)
    W2 = np.asarray(W2, dtype=np.float32)
    b2 = np.asarray(b2, dtype=np.float32)
    b = np.asarray(batch).astype(np.int64)

    E, D = src.shape
    G = u.shape[0]
    assert D == P and E % N_CORES == 0
    E0 = E // N_CORES
    CW = CH * TILE
    Epad = ((E0 + CW - 1) // CW) * CW
    T = Epad // TILE

    # Fold u[batch] @ W1d + b1 into a per-group table (tiny: G x D).
    z = (u @ W1[3 * D:4 * D] + b1).astype(np.float32)  # [G, D]

    # Per-core: tile-local group offsets for the z-selection matmul.
    g0s, js = [], []
    k_s = 1
    for c in range(N_CORES):
        bc = b[c * E0:(c + 1) * E0]
        bp = np.concatenate([bc, np.full(Epad - E0, bc[-1], dtype=np.int64)])
        per_tile = bp.reshape(T, TILE)
        g0 = per_tile.min(axis=1)                 # [T]
        j = bp - np.repeat(g0, TILE)              # [Epad], >= 0
        g0s.append(g0)
        js.append(j)
        k_s = max(k_s, int(j.max()) + 1)

    mmdt, sdt = _np_dtypes()
    in_maps = []
    w1_in = np.ascontiguousarray(W1[:3 * D]).astype(mmdt)
    w2_in = np.ascontiguousarray(W2).astype(mmdt)
    b2_in = np.ascontiguousarray(b2.reshape(P, 1))
    for c in range(N_CORES):
        sl = slice(c * E0, (c + 1) * E0)

        def tr(x, dt):
            out = np.zeros((P, Epad), dtype=dt)
            out[:, :E0] = x[sl].T.astype(dt)
            return out

        selc = np.zeros((k_s, Epad), dtype=sdt)
        selc[js[c], np.arange(Epad)] = 1.0
        selc[:, E0:] = 0.0  # pad edges contribute nothing
        gidx = np.clip(g0s[c][:, None] + np.arange(k_s)[None, :], 0, G - 1)
        zwc = np.ascontiguousarray(
            z[gidx].transpose(1, 0, 2).reshape(k_s, T * P)).astype(mmdt)
        in_maps.append({
            "srcT": tr(src, sdt), "destT": tr(dest, mmdt),
            "eaT": tr(edge_attr, mmdt),
            "w1": w1_in, "w2": w2_in, "b2c": b2_in,
            "sel": selc, "zw": zwc,
        })

    profile = os.environ.get("KERNEL_PROFILE", "") == "1"
    if profile:
        _install_profile_shim()

    nc = _get_program(T, k_s)
    from concourse.bass_utils import run_bass_kernel_spmd
    kwargs = {}
    if profile:
        kwargs["trace"] = True
        if os.environ.get("KERNEL_PROFILE_ALL", "") == "1":
            kwargs["trace_cores"] = list(range(N_CORES))
    res = run_bass_kernel_spmd(nc, in_maps, core_ids=list(range(N_CORES)),
                               **kwargs)
    if profile and res.exec_time_ns is not None:
        with open("/tmp/kernel_exec_ns.txt", "w") as f:
            f.write(str(res.exec_time_ns))
        print(f"HW exec time: {res.exec_time_ns} ns")

    out = np.empty((E, P), dtype=np.float32)
    for c in range(N_CORES):
        out[c * E0:(c + 1) * E0] = \
            res.results[c]["outT"][:, :E0].T.astype(np.float32)
    return out


# revision 7
# speedup vs baseline: 2.2987x; 1.0525x over previous
# Trainium2 Bass kernel for the MEGNet edge model:
#   out = relu(concat([src, dest, edge_attr, u[batch]], 1) @ W1 + b1) @ W2 + b2
#
# Strategy (8 NeuronCores, SPMD, edges sharded contiguously):
#  * All tensors are shipped in a transposed, feature-major layout [128, E_pad]
#    so the PE array contracts over features with no on-chip transposes; the
#    host transposes shards on the way in and the output on the way out.
#  * comb @ W1 decomposes into src@W1a + dest@W1b + edge_attr@W1c +
#    u[batch]@W1d.  The u[batch] term plus b1 folds into a per-group table
#    z = u @ W1d + b1 [G, 128]; batch is sorted, so each 512-edge tile spans
#    only a few consecutive groups and z[batch] is applied with one small
#    matmul per tile (lhsT = the k_s candidate z-rows, rhs = a one-hot
#    selection matrix built on the host).
#  * Traffic is the roofline: dest/edge_attr/output ship as bf16, src and the
#    one-hot sel matrix as fp8e4 (sel is 0/1, exact in fp8).  Total rel err
#    ~1.5e-2 vs the 2e-2 gate (validated numerically against the reference).
#  * The W2 matmuls run one tile-pair behind the accumulation matmuls so the
#    PE never stalls waiting for the ReLU; weights/zws/sel for the first
#    chunks load via the hardware-DGE queues (the gpsimd software queue takes
#    ~10us to produce its first descriptor batch).
import os
import numpy as np

N_CORES = 8
P = 128      # feature dim == SBUF partitions
TILE = 512   # edges per matmul tile (one PSUM bank of fp32)
CH = 14      # max matmul tiles per DMA chunk (pool slot size)
# chunk sizes: smaller leading chunks let compute start sooner
CHUNK_SIZES = [4, 6, 8, 10] + [14] * 5   # sums to 98 tiles

MM_DTYPE = os.environ.get("KERNEL_MM_DTYPE", "bf16")
SRC_FP8 = os.environ.get("KERNEL_SRC_FP8", "1") == "1"
OUT_BF16 = os.environ.get("KERNEL_OUT_BF16", "1") == "1"

_prog_cache = {}


def _np_dtypes():
    import ml_dtypes
    mm = {"f32": np.float32, "f32r": np.float32,
          "bf16": ml_dtypes.bfloat16}[MM_DTYPE]
    sdt = ml_dtypes.float8_e4m3 if SRC_FP8 else mm
    return mm, sdt


def _build_program(T, k_s):
    import concourse.bacc as bacc
    import concourse.tile as tile
    from concourse import mybir

    f32 = mybir.dt.float32
    mdt = {"f32": mybir.dt.float32, "f32r": mybir.dt.float32r,
           "bf16": mybir.dt.bfloat16}[MM_DTYPE]
    sdt = mybir.dt.float8e4 if SRC_FP8 else mdt
    odt = mybir.dt.bfloat16 if OUT_BF16 else f32
    Relu = mybir.ActivationFunctionType.Relu
    Epad = T * TILE

    nc = bacc.Bacc("TRN2", target_bir_lowering=False, debug=False,
                   num_devices=N_CORES)
    srcT = nc.dram_tensor("srcT", [P, Epad], sdt, kind="ExternalInput")
    destT = nc.dram_tensor("destT", [P, Epad], mdt, kind="ExternalInput")
    eaT = nc.dram_tensor("eaT", [P, Epad], mdt, kind="ExternalInput")
    wpkd = nc.dram_tensor("wpk", [P, 4 * P], mdt, kind="ExternalInput")
    b2d = nc.dram_tensor("b2c", [P, 1], f32, kind="ExternalInput")
    seld = nc.dram_tensor("sel", [k_s, Epad], sdt, kind="ExternalInput")
    zwd = nc.dram_tensor("zw", [k_s, T * P], mdt, kind="ExternalInput")
    outT = nc.dram_tensor("outT", [P, Epad], odt, kind="ExternalOutput")

    assert sum(CHUNK_SIZES) == T
    CW = CH * TILE  # max chunk width in edges (pool slot size)

    with tile.TileContext(nc) as tc:
        with (
            tc.tile_pool(name="const", bufs=1) as constp,
            tc.tile_pool(name="inp", bufs=3) as inp,
            tc.tile_pool(name="hp", bufs=6) as hp,
            tc.tile_pool(name="outp", bufs=4) as outp,
            tc.tile_pool(name="ps1", bufs=4, space="PSUM") as ps1,
            tc.tile_pool(name="ps2", bufs=4, space="PSUM") as ps2,
        ):
            wpk = constp.tile([P, 4 * P], mdt, tag="wpk", name="wpk")
            b2s = constp.tile([P, 1], f32, tag="b2s", name="b2s")
            zws = constp.tile([k_s, T * P], mdt, tag="zws", name="zws")
            nc.sync.dma_start(wpk[:], wpkd[:])
            nc.sync.dma_start(b2s[:], b2d[:])
            nc.scalar.dma_start(zws[:], zwd[:])
            w1a = wpk[:, 0:P]
            w1b = wpk[:, P:2 * P]
            w1c = wpk[:, 2 * P:3 * P]
            w2s = wpk[:, 3 * P:4 * P]

            # Software-pipelined tile stream: accumulate a tile pair, then
            # issue the W2 matmuls of the previous pair (whose ReLUs ran
            # during this pair's accumulation matmuls).  Output DMAs are
            # likewise delayed so the last vector add into an output tile
            # lands before its DMA is issued.
            pend = []        # [(h_tile, ot_tile, col_slice_in_ot, tile_idx)]
            pend_out = None  # (dram_lo, dram_hi, ot_tile, ow, queue_parity)

            def flush_pending():
                nonlocal pend
                for h, ot, ocs, ti in pend:
                    p2 = ps2.tile([P, TILE], f32, tag="p2", name=f"p2_{ti}")
                    nc.tensor.matmul(p2[:], w2s, h[:], start=True, stop=True)
                    nc.vector.tensor_scalar_add(ot[:, ocs], p2[:], b2s[:])
                pend = []

            def issue_pend_out():
                nonlocal pend_out
                if pend_out is None:
                    return
                lo, hi, ot, ow, par = pend_out
                eng = nc.scalar if par == 0 else nc.sync
                eng.dma_start(outT[:, lo:hi], ot[:, :ow])
                pend_out = None

            t = 0
            for ci, csz in enumerate(CHUNK_SIZES):
                base = t * TILE
                cw = csz * TILE
                st = inp.tile([P, CW], sdt, tag="src", name=f"st{ci}")
                dt = inp.tile([P, CW], mdt, tag="dest", name=f"dt{ci}")
                et = inp.tile([P, CW], mdt, tag="ea", name=f"et{ci}")
                slt = inp.tile([k_s, CW], sdt, tag="sel", name=f"slt{ci}")
                if ci % 2 == 0:
                    nc.scalar.dma_start(st[:, :cw], srcT[:, base:base + cw])
                else:
                    nc.sync.dma_start(st[:, :cw], srcT[:, base:base + cw])
                nc.sync.dma_start(dt[:, :cw], destT[:, base:base + cw])
                nc.scalar.dma_start(et[:, :cw], eaT[:, base:base + cw])
                if ci < 2:  # gpsimd's software DGE takes ~10us to spin up
                    nc.sync.dma_start(slt[:, :cw], seld[:, base:base + cw])
                else:
                    nc.gpsimd.dma_start(slt[:, :cw], seld[:, base:base + cw])

                # output tiles per half-chunk for a short drain tail
                nho = (csz + 6) // 7
                for ho in range(nho):
                    o0 = ho * 7 * TILE
                    ow = min(7 * TILE, cw - o0)
                    ot = outp.tile([P, 7 * TILE], odt, tag="o",
                                   name=f"ot{ci}_{ho}")
                    tl0 = ho * 7
                    tln = min(ho * 7 + 7, csz)
                    tl = tl0
                    while tl < tln:
                        npair = min(2, tln - tl)
                        p1s = []
                        for i in range(npair):
                            p1s.append(ps1.tile([P, TILE], f32, tag="p1",
                                                name=f"p1_{t + i}"))
                        # shared-stationary accumulation matmuls
                        for w, buf in ((w1a, st), (w1b, dt), (w1c, et)):
                            for i in range(npair):
                                cs = slice((tl + i) * TILE,
                                           (tl + i + 1) * TILE)
                                nc.tensor.matmul(p1s[i][:], w, buf[:, cs],
                                                 start=(w is w1a),
                                                 stop=False)
                        for i in range(npair):
                            ti = t + i
                            cs = slice((tl + i) * TILE, (tl + i + 1) * TILE)
                            for j0 in range(0, k_s, P):
                                j1 = min(j0 + P, k_s)
                                nc.tensor.matmul(
                                    p1s[i][:],
                                    zws[j0:j1, ti * P:(ti + 1) * P],
                                    slt[j0:j1, cs],
                                    start=False, stop=(j1 == k_s))
                        flush_pending()
                        if tl == tl0:
                            issue_pend_out()
                        for i in range(npair):
                            ti = t + i
                            h = hp.tile([P, TILE], mdt, tag="h",
                                        name=f"h{ti}")
                            nc.scalar.activation(h[:], p1s[i][:], Relu)
                            ocs = slice((tl + i) * TILE - o0,
                                        (tl + i + 1) * TILE - o0)
                            pend.append((h, ot, ocs, ti))
                        t += npair
                        tl += npair

                    pend_out = (base + o0, base + o0 + ow, ot, ow, ci % 2)
            flush_pending()
            issue_pend_out()

    nc.compile()
    return nc


def _get_program(T, k_s):
    key = (T, k_s)
    if key not in _prog_cache:
        _prog_cache[key] = _build_program(T, k_s)
    return _prog_cache[key]


def _install_profile_shim():
    """Optional: enable NTFF profiling under axon (KERNEL_PROFILE=1)."""
    import sys, types
    if "antenv.axon_hooks" not in sys.modules:
        mod = types.ModuleType("antenv.axon_hooks")
        mod._hook = None
        mod.set_axon_ntff_profile_hook = lambda h: setattr(mod, "_hook", h)
        mod.get_axon_ntff_profile_hook = lambda: mod._hook
        sys.modules["antenv.axon_hooks"] = mod
        try:
            import antenv
            antenv.axon_hooks = mod
        except ImportError:
            pass
        try:
            from trn_agent_boot.trn_boot import _ntff_profile_via_ctypes
            mod.set_axon_ntff_profile_hook(
                _ntff_profile_via_ctypes("/opt/axon/libaxon_pjrt.so"))
        except Exception:
            pass
    import concourse.bass_utils as bass_utils
    bass_utils.upload_artifacts = lambda tmpdir: tmpdir


def kernel(src, dest, edge_attr, u, batch, W1, b1, W2, b2):
    src = np.asarray(src, dtype=np.float32)
    dest = np.asarray(dest, dtype=np.float32)
    edge_attr = np.asarray(edge_attr, dtype=np.float32)
    u = np.asarray(u, dtype=np.float32)
    W1 = np.asarray(W1, dtype=np.float32)
    b1 = np.asarray(b1, dtype=np.float32)
    W2 = np.asarray(W2, dtype=np.float32)
    b2 = np.asarray(b2, dtype=np.float32)
    b = np.asarray(batch).astype(np.int64)

    E, D = src.shape
    G = u.shape[0]
    assert D == P and E % N_CORES == 0
    E0 = E // N_CORES
    CW = CH * TILE
    Epad = ((E0 + CW - 1) // CW) * CW
    T = Epad // TILE

    # Fold u[batch] @ W1d + b1 into a per-group table (tiny: G x D).
    z = (u @ W1[3 * D:4 * D] + b1).astype(np.float32)  # [G, D]

    # Per-core: tile-local group offsets for the z-selection matmul.
    g0s, js = [], []
    k_s = 1
    for c in range(N_CORES):
        bc = b[c * E0:(c + 1) * E0]
        bp = np.concatenate([bc, np.full(Epad - E0, bc[-1], dtype=np.int64)])
        per_tile = bp.reshape(T, TILE)
        g0 = per_tile.min(axis=1)                 # [T]
        j = bp - np.repeat(g0, TILE)              # [Epad], >= 0
        g0s.append(g0)
        js.append(j)
        k_s = max(k_s, int(j.max()) + 1)

    mmdt, sdt = _np_dtypes()
    in_maps = []
    wpk_in = np.concatenate(
        [W1[0 * D:1 * D], W1[1 * D:2 * D], W1[2 * D:3 * D], W2],
        axis=0).reshape(4, D, D).transpose(1, 0, 2).reshape(D, 4 * D)
    wpk_in = np.ascontiguousarray(wpk_in).astype(mmdt)
    b2_in = np.ascontiguousarray(b2.reshape(P, 1))
    for c in range(N_CORES):
        sl = slice(c * E0, (c + 1) * E0)

        def tr(x, dt):
            out = np.zeros((P, Epad), dtype=dt)
            out[:, :E0] = x[sl].T.astype(dt)
            return out

        selc = np.zeros((k_s, Epad), dtype=sdt)
        selc[js[c], np.arange(Epad)] = 1.0
        selc[:, E0:] = 0.0  # pad edges contribute nothing
        gidx = np.clip(g0s[c][:, None] + np.arange(k_s)[None, :], 0, G - 1)
        zwc = np.ascontiguousarray(
            z[gidx].transpose(1, 0, 2).reshape(k_s, T * P)).astype(mmdt)
        in_maps.append({
            "srcT": tr(src, sdt), "destT": tr(dest, mmdt),
            "eaT": tr(edge_attr, mmdt),
            "wpk": wpk_in, "b2c": b2_in,
            "sel": selc, "zw": zwc,
        })

    profile = os.environ.get("KERNEL_PROFILE", "") == "1"
    if profile:
        _install_profile_shim()

    nc = _get_program(T, k_s)
    from concourse.bass_utils import run_bass_kernel_spmd
    kwargs = {}
    if profile:
        kwargs["trace"] = True
        if os.environ.get("KERNEL_PROFILE_ALL", "") == "1":
            kwargs["trace_cores"] = list(range(N_CORES))
    res = run_bass_kernel_spmd(nc, in_maps, core_ids=list(range(N_CORES)),
                               **kwargs)
    if profile and res.exec_time_ns is not None:
        with open("/tmp/kernel_exec_ns.txt", "w") as f:
            f.write(str(res.exec_time_ns))
        print(f"HW exec time: {res.exec_time_ns} ns")

    out = np.empty((E, P), dtype=np.float32)
    for c in range(N_CORES):
        out[c * E0:(c + 1) * E0] = \
            res.results[c]["outT"][:, :E0].T.astype(np.float32)
    return out
